# revision 57
# baseline (speedup 1.0000x reference)
"""Bass/Tile TRN2 kernel for nn_Attention_26388279067013 (v3).

Computes, for each batch row b:
    feat = enc @ We.T + dec @ Ws.T + cov[:,None] * Wc.sum(1) + b     [S, H]
    att  = tanh(feat) @ v_w                                          [S]
    att[s >= L_b] = -inf ; w = softmax(att) ; new_cov = cov + w

Key optimizations over the f32r baseline (260us -> ~104us):
  - enc/We in fp8 e4m3 (x16 / x64 scaling); the 1/1024 descale rides
    the tanh activation's free scale. DMA traffic drops 4x.
  - hybrid DoubleRow: 2 of every 3 s-tiles use 2 virtual-K=256 DR
    matmuls (0.5 cyc/row); every 3rd stays plain fp8 (4 matmuls)
    because DR matmuls are invisible to the PE clock governor (HAM) --
    an all-DR stream gets stuck at K=4/8 half clock (measured), while
    a ~45% plain duty cycle keeps the array at 2.4 GHz.
  - dec/bias/coverage rank-1 terms via one bf16 matmul per s-tile
    ([ones; cov] x [db; wc_sum], db = dec @ Ws.T + b computed on
    host), zero-padded to K=128: a 2-row stationary defeats the
    LDWEIGHTS pull-ahead and costs ~2x the stream time in stalls.
  - masked positions (s >= L_b) have w == 0 exactly, so only
    ceil(L/128) s-tiles are computed. Batches are sorted by length and
    dealt round-robin to (core, slot) so the compiled per-slot tile
    counts (max over cores) stay small; host fills w=0 / c=cov for the
    skipped tail. new_cov = cov + w is a host-side add.
  - x = tanh(feat) in bf16 -> DVE scalar_tensor_tensor v-dot.
  - softmax split into a DVE/ACT stage and a PE stage emitted a chunk
    later, so the strict PE queue never blocks on the exp.

Sharding: 4 batch slots per core across 8 NeuronCores (SPMD).
"""

import sys

sys.path.insert(0, "/opt/trn_rl_repo")

import numpy as np
import ml_dtypes

import concourse.bacc as bacc
import concourse.tile as tile
import concourse.mybir as mybir
from concourse.bass_utils import run_bass_kernel_spmd

B, S, H, D = 32, 4096, 512, 256
N_CORES = 8
N_SLOTS = 4
F32 = mybir.dt.float32
BF16 = mybir.dt.bfloat16
F8 = mybir.dt.float8e4
ALU = mybir.AluOpType
ACTF = mybir.ActivationFunctionType
DR = mybir.MatmulPerfMode.DoubleRow
NP_F8 = ml_dtypes.float8_e4m3
NP_BF = ml_dtypes.bfloat16

SE = 16.0                     # enc fp8 scale
SW = 64.0                     # We fp8 scale
SCALE = SE * SW               # psum arrives x1024; tanh descales
NEG_BIG = -30000.0            # exp(x - 30000) == 0.0 exactly in f32
CHUNK = 3                     # s-tiles per psum tile (3 banks of 4KiB)
GRP = 6                       # s-tiles per contiguous enc DMA group
DR_EVERY = 4                  # of every DR_EVERY s-tiles, DR_EVERY-1 use
                              # DoubleRow and one stays plain fp8 (DR is
                              # invisible to the PE clock governor, so plain
                              # tiles must keep feeding it); 0 = all plain


def build_kernel(tiles):
    """tiles: per-slot s-tile counts (max over cores), e.g. (32, 27, 20, 10)."""
    nc = bacc.Bacc("TRN2", debug=False, num_devices=N_CORES)

    # enc packed per group of GRP s-tiles: [p, (g, k, s)] so each group is
    # one contiguous ~384KB DMA (fast first arrival, clean prefetch).
    grps = [(t + GRP - 1) // GRP for t in tiles]
    enc_d = [
        nc.dram_tensor(f"enc8_{s}", [128, g * 4 * GRP * 128], F8,
                       kind="ExternalInput").ap()
        for s, g in zip(range(N_SLOTS), grps)
    ]
    cov_d = [
        nc.dram_tensor(f"cov_{s}", [2, t * 128], BF16, kind="ExternalInput").ap()
        for s, t in enumerate(tiles)
    ]
    aug_d = [
        nc.dram_tensor(f"aug_{s}", [2, H], BF16, kind="ExternalInput").ap()
        for s in range(N_SLOTS)
    ]
    we_d = nc.dram_tensor("we8", [128, 4 * H], F8, kind="ExternalInput").ap()
    v_d = nc.dram_tensor("v_row", [1, H], BF16, kind="ExternalInput").ap()
    lens_d = nc.dram_tensor("lens", [N_SLOTS, 1], F32, kind="ExternalInput").ap()
    iota_d = nc.dram_tensor("iota_pm", [128, 32], F32, kind="ExternalInput").ap()
    ident_d = nc.dram_tensor("ident", [128, 128], F32, kind="ExternalInput").ap()
    out_d = [
        nc.dram_tensor(f"out_w_{s}", [t, 128], F32, kind="ExternalOutput").ap()
        for s, t in enumerate(tiles)
    ]

    with tile.TileContext(nc) as tc:
        with (
            tc.tile_pool(name="persist", bufs=1) as pp,
            tc.tile_pool(name="x", bufs=3) as xp,
            tc.tile_pool(name="scratch", bufs=2) as scrp,
            tc.tile_pool(name="small", bufs=4) as smp,
            tc.tile_pool(name="batch", bufs=3) as bp,
            tc.tile_pool(name="psum", bufs=2, space="PSUM") as psp,
            tc.tile_pool(name="psum_misc", bufs=2, space="PSUM") as psm,
        ):
            # ---- one-time setup ----
            # we8 and the enc groups all go on the sync HWDGE ring in
            # priority order: ring FIFO guarantees the first-needed ~640KB
            # transfers before the remaining ~4MB instead of sharing SDMA
            # bandwidth with it (first matmul ~7us instead of ~12us).
            we_t = pp.tile([128, 4 * H], F8, tag="we8")
            nc.sync.dma_start(we_t[:], we_d[:, :])
            vrow_sb = pp.tile([1, H], BF16, tag="vrow")
            nc.sync.dma_start(vrow_sb[:], v_d[:, :])
            iota_sb = pp.tile([128, 32], F32, tag="iota")
            nc.sync.dma_start(iota_sb[:], iota_d[:, :])
            ident_sb = pp.tile([128, 128], F32, tag="ident")
            nc.sync.dma_start(ident_sb[:], ident_d[:, :])
            # big slots first: long uninterrupted streams while the HAM
            # warms; the small final slot keeps the tail short.
            slot_order = sorted(range(N_SLOTS), key=lambda s: -tiles[s])
            enc_sb = [None] * N_SLOTS
            GW = 4 * GRP * 128
            for s in range(N_SLOTS):
                enc_t = pp.tile([128, grps[s] * GW], F8, tag=f"enc{s}")
                enc_sb[s] = enc_t
            for s in slot_order:
                for g in range(grps[s]):
                    nc.sync.dma_start(
                        enc_sb[s][:, g * GW:(g + 1) * GW],
                        enc_d[s][:, g * GW:(g + 1) * GW])

            ones_k1 = pp.tile([1, 128], F32, tag="ones_k1")
            nc.vector.memset(ones_k1[:], 1.0)
            ones_col = pp.tile([128, 1], F32, tag="ones_col")
            nc.vector.memset(ones_col[:], 1.0)
            ones_bf = pp.tile([1, 128], BF16, tag="ones_bf")
            nc.vector.memset(ones_bf[:], 1.0)

            # aug operands padded to K=128 (rows 2-127 zero) so the aug
            # matmul's LDWEIGHTS overlaps the preceding stream like the
            # full-K enc matmuls do (a 2-row stationary defeats the
            # weight-load pull-ahead). Double-buffered across slots.
            aug_lhs, aug_rhs = [], []
            for i in range(2):
                t = pp.tile([128, 32 * 128], BF16, tag=f"auglhs{i}")
                nc.vector.memset(t[:], 0.0)
                aug_lhs.append(t)
                t = pp.tile([128, H], BF16, tag=f"augrhs{i}")
                nc.vector.memset(t[:], 0.0)
                aug_rhs.append(t)





            # v_bcast[p, o] = v_w[o]  (bf16 for the 2x DVE v-dot)
            ps_vb = psm.tile([128, H], F32, tag="mpsum")
            nc.tensor.matmul(ps_vb[:], ones_bf[:], vrow_sb[:],
                             start=True, stop=True)
            v_bcast = pp.tile([128, H], BF16, tag="v_bcast")
            nc.scalar.copy(v_bcast[:], ps_vb[:])

            state = {}

            def emit_prep(s):
                nt = tiles[s]
                cov_sb = aug_lhs[s % 2]
                nc.gpsimd.dma_start(cov_sb[0:2, :nt * 128], cov_d[s])
                aug_sb = aug_rhs[s % 2]
                nc.gpsimd.dma_start(aug_sb[0:2, :], aug_d[s])
                len_sb = smp.tile([1, 1], F32, tag="len_sb")
                nc.gpsimd.dma_start(len_sb[:], lens_d[s:s + 1, :])
                ps_l = psm.tile([128, 1], F32, tag="mpsum")
                nc.tensor.matmul(ps_l[:], ones_k1[:], len_sb[:],
                                 start=True, stop=True)
                l_col = smp.tile([128, 1], F32, tag="l_col")
                nc.vector.tensor_scalar(l_col[:], ps_l[:], 1.0, None, ALU.mult)
                att_pm = bp.tile([128, 32], F32, tag="att_pm")
                state[s] = dict(cov=cov_sb, aug=aug_sb, l_col=l_col,
                                att_pm=att_pm)

            def emit_chunk(s, t0, ntile):
                st8 = state[s]
                ps = psp.tile([128, CHUNK * 512], F32, tag="feat")
                enc_ap = enc_sb[s][:].rearrange(
                    "p (g k q) -> p g k q", g=grps[s], k=4)
                we_ap = we_t[:].rearrange("p (k q) -> p k q", k=4)
                for j in range(ntile):
                    t = t0 + j
                    g, jj = t // GRP, (t % GRP) * 128
                    dst = ps[:, j * 512:(j + 1) * 512]
                    if DR_EVERY and t % DR_EVERY != 0:
                        for k2 in range(2):
                            nc.tensor.matmul(
                                dst,
                                enc_ap[:, g, 2 * k2:2 * k2 + 2,
                                       jj:jj + 128],
                                we_ap[:, 2 * k2:2 * k2 + 2, :],
                                start=(k2 == 0), stop=False, perf_mode=DR)
                    else:
                        for k in range(4):
                            nc.tensor.matmul(
                                dst, enc_ap[:, g, k, jj:jj + 128],
                                we_ap[:, k, :], start=(k == 0), stop=False)
                    nc.tensor.matmul(
                        dst, st8["cov"][:, t * 128:(t + 1) * 128],
                        st8["aug"][:, :], start=False, stop=True)
                x = xp.tile([128, CHUNK * 512], BF16, tag="x")
                nc.scalar.activation(x[:, :ntile * 512], ps[:, :ntile * 512],
                                     ACTF.Tanh, scale=1.0 / SCALE)
                for j in range(ntile):
                    t = t0 + j
                    scr = scrp.tile([128, 512], BF16, tag="vscr")
                    nc.vector.scalar_tensor_tensor(
                        scr[:], x[:, j * 512:(j + 1) * 512],
                        1.0, v_bcast[:], ALU.bypass, ALU.mult,
                        accum_out=st8["att_pm"][:, t:t + 1])

            def emit_softmax_a(s):
                st8 = state[s]
                nt = tiles[s]
                att_pm, l_col = st8["att_pm"], st8["l_col"]
                pad01 = bp.tile([128, 32], F32, tag="pad01")
                nc.vector.tensor_scalar(pad01[:, :nt], iota_sb[:, :nt],
                                        l_col[:], None, ALU.is_ge)
                att_m = bp.tile([128, 32], F32, tag="att_m")
                nc.vector.scalar_tensor_tensor(
                    att_m[:, :nt], pad01[:, :nt], NEG_BIG, att_pm[:, :nt],
                    ALU.mult, ALU.add)
                exp_pm = bp.tile([128, 32], F32, tag="exp_pm")
                rowsum = smp.tile([128, 1], F32, tag="rowsum")
                nc.scalar.activation(exp_pm[:, :nt], att_m[:, :nt], ACTF.Exp,
                                     accum_out=rowsum[:])
                st8["exp_pm"] = exp_pm
                st8["rowsum"] = rowsum

            def emit_softmax_b(s):
                st8 = state.pop(s)
                nt = tiles[s]
                exp_pm, rowsum = st8["exp_pm"], st8["rowsum"]
                ps_d = psm.tile([1, 1], F32, tag="mpsum")
                nc.tensor.matmul(ps_d[:], rowsum[:], ones_col[:],
                                 start=True, stop=True)
                rinv = smp.tile([1, 1], F32, tag="rinv")
                nc.vector.reciprocal(rinv[:], ps_d[:])
                ps_r = psm.tile([128, 1], F32, tag="mpsum")
                nc.tensor.matmul(ps_r[:], ones_k1[:], rinv[:],
                                 start=True, stop=True)
                rinv_col = smp.tile([128, 1], F32, tag="rinv_col")
                nc.vector.tensor_scalar(rinv_col[:], ps_r[:], 1.0, None,
                                        ALU.mult)
                w_pm = bp.tile([128, 32], F32, tag="w_pm")
                nc.vector.tensor_scalar(w_pm[:, :nt], exp_pm[:, :nt],
                                        rinv_col[:], None, ALU.mult)
                ps_t = psm.tile([32, 128], F32, tag="mpsum")
                nc.tensor.transpose(ps_t[:nt, :], w_pm[:, :nt], ident_sb[:])
                w_sb = bp.tile([32, 128], F32, tag="w_sb")
                nc.vector.tensor_scalar(w_sb[:nt, :], ps_t[:nt, :], 1.0, None,
                                        ALU.mult)
                nc.sync.dma_start(out_d[s], w_sb[:nt, :])

            # chunk schedule: list of (slot, t0, ntile), slots small->big
            sched = []
            for s in slot_order:
                nt = tiles[s]
                for t0 in range(0, nt, CHUNK):
                    sched.append((s, t0, min(CHUNK, nt - t0)))

            emit_prep(slot_order[0])
            emit_prep(slot_order[1])
            order_pos = {s: i for i, s in enumerate(slot_order)}
            # softmax is split: stage a (DVE/ACT) at the slot transition,
            # before the next slot's first chunk hits the ACT queue; stage
            # b (PE-dependent chain) one chunk later so the strict PE
            # queue never waits on the exp.
            pend_b = []
            prev_slot = slot_order[0]
            for ci, (s, t0, ntile) in enumerate(sched):
                if s != prev_slot:
                    nxt = order_pos[s] + 1
                    if nxt < N_SLOTS:
                        emit_prep(slot_order[nxt])
                    emit_softmax_a(prev_slot)
                    pend_b.append((ci + 1, prev_slot))
                    prev_slot = s
                emit_chunk(s, t0, ntile)
                if pend_b and pend_b[0][0] == ci:
                    _, ps_ = pend_b.pop(0)
                    emit_softmax_b(ps_)
            for _, ps_ in pend_b:
                emit_softmax_b(ps_)
            emit_softmax_a(slot_order[-1])
            emit_softmax_b(slot_order[-1])

    nc.compile()
    return nc


_NC_CACHE = {}


def _get_nc(tiles):
    key = tuple(tiles)
    if key not in _NC_CACHE:
        _NC_CACHE[key] = build_kernel(key)
    return _NC_CACHE[key]


def kernel(dec_input, enc_output, coverage_vector, text_lengths, W, b, v_w, v_b,
           _trace=False):
    dec_input = np.asarray(dec_input, np.float32)
    enc_output = np.asarray(enc_output, np.float32)
    coverage_vector = np.asarray(coverage_vector, np.float32)
    lens = np.asarray(text_lengths).astype(np.int64)
    W = np.asarray(W, np.float32)
    b = np.asarray(b, np.float32)
    v_w = np.asarray(v_w, np.float32)

    We = W[:, :H]
    Ws = W[:, H:H + D]
    Wc = W[:, H + D:]
    wc_sum = Wc.sum(axis=1)
    db = dec_input[:, 0, :] @ Ws.T + b          # [B, H] host GEMV (tiny)

    # deal batches to (core, slot) by length rank: slot s takes ranks
    # [8s, 8s+8), so the compiled per-slot cap is the max in that octet.
    order = np.argsort(-lens, kind="stable")
    assign = order.reshape(N_SLOTS, N_CORES)     # [slot, core] -> batch
    tiles = tuple(
        int(np.ceil(lens[assign[s]].max() / 128.0)) for s in range(N_SLOTS)
    )

    nc = _get_nc(tiles)

    we8 = np.ascontiguousarray(
        (We.T * SW).astype(NP_F8).reshape(4, 128, H).transpose(1, 0, 2)
        .reshape(128, 4 * H))
    iota_pm = (np.arange(32)[None, :] * 128
               + np.arange(128)[:, None]).astype(np.float32)
    ident = np.eye(128, dtype=np.float32)
    v_bf = np.ascontiguousarray(v_w[None, :].astype(NP_BF))

    in_maps = []
    for core in range(N_CORES):
        m = {"we8": we8, "v_row": v_bf, "iota_pm": iota_pm, "ident": ident}
        lens_f = np.zeros((N_SLOTS, 1), np.float32)
        for s in range(N_SLOTS):
            bidx = int(assign[s, core])
            nt = tiles[s]
            sp = nt * 128
            lens_f[s, 0] = lens[bidx]
            g = (nt + 5) // 6
            e8 = np.zeros((g * 768, 512), NP_F8)
            e8[:sp] = (enc_output[bidx, :sp, :] * SE).astype(NP_F8)
            m[f"enc8_{s}"] = np.ascontiguousarray(
                e8.reshape(g, 768, 4, 128).transpose(3, 0, 2, 1)
                .reshape(128, g * 3072))
            cov_aug = np.ones((2, sp), np.float32)
            cov_aug[1] = coverage_vector[bidx, :sp]
            m[f"cov_{s}"] = cov_aug.astype(NP_BF)
            aug = np.stack([db[bidx] * SCALE, wc_sum * SCALE])
            m[f"aug_{s}"] = aug.astype(NP_BF)
        m["lens"] = lens_f
        in_maps.append(m)

    res = run_bass_kernel_spmd(nc, in_maps, list(range(N_CORES)), trace=_trace)

    w = np.zeros((B, S), np.float32)
    for core in range(N_CORES):
        for s in range(N_SLOTS):
            bidx = int(assign[s, core])
            sp = tiles[s] * 128
            w[bidx, :sp] = res.results[core][f"out_w_{s}"].reshape(-1)
    c = coverage_vector + w
    if _trace:
        kernel.last_result = res
    return w, c
